# revision 28
# baseline (speedup 1.0000x reference)
"""Trainium2 Bass kernel for nn_AccumulatingModule (histogram_binning).

Problem: out = score_matrix.at[qt, p, ol1, ol2].add(at1*at2) — a scatter-add of
BATCH*PAIR outer-product contributions into a [65, 90, 151, 151] fp32 histogram.

Strategy (8 NeuronCores, SPMD):
  * Memory roofline: stream score_matrix (533 MB) in + out once; everything
    else must hide under that.
  * Shard the (qt, pair) space: each qt's 90 pairs split into two 45-pair
    "half sections" (first-box-index i in {0..4} / {5..9}); 130 half
    sections + 6 dummies = 17 per core.
  * Box-permutation trick keeps the compiled kernel identical across cores
    (SPMD): every section computes the FIXED pattern pairs {(i,j): i in
    0..4, j != i}; the host permutes the 10 box columns per section and
    orders score rows to match the kernel's slot order.
  * W[b,k,:] = attention[b,k] * onehot(label[b,k]) built on GpSimd
    (tensor_scalar is_equal*mult vs an iota row), bf16 (one-hot exact, at
    rounded once -> ~2^-9 relative error on the sparse delta only).
  * delta[pair(i,j)] = W_j^T @ W_i on TensorE, PSUM-accumulated over two
    128-row chunks.  o1=151 -> 128-row main piece + 23-row tail; tails of
    4 groups share one PSUM bank at partition offsets 0/32/64/96 via
    matmul col-tiling.
  * The host pre-swizzles score into partition-major DRAM blocks
    (score_main [128, slots*151], score_tail [128, tailw] banded) so every
    score transfer is a 2D full-128-partition DMA — few dma_starts (HWDGE
    is a ~625ns/DMA serial resource) with large per-partition descriptors.
  * out = psum + score on VectorE, DMA'd back to the swizzled layout; host
    un-swizzles and scatters rows back.
"""

import numpy as np

NUM_QT, NUM_OT, PAIR = 65, 151, 90
BOX = 10
OT = NUM_OT
ROWLEN = OT * OT  # 22801
SECP = 45  # pairs per (half) section
NSEC = 17  # sections per core
NCORES = 8
ROWS_PER_SEC = 256  # padded batch rows per section (2 chunks of 128)
MAIN_W = SECP * OT  # main free width per section (per partition)


def _pattern_groups():
    """(j, istart, gsize) groups of pattern pairs; consecutive moving i."""
    groups = []
    for j in range(BOX):
        ilist = [i for i in range(5) if i != j]
        runs = []
        cur = [ilist[0]]
        for i in ilist[1:]:
            if i == cur[-1] + 1:
                cur.append(i)
            else:
                runs.append(cur)
                cur = [i]
        runs.append(cur)
        for run in runs:
            for cs in range(0, len(run), 3):
                chunk = run[cs : cs + 3]
                groups.append((j, chunk[0], len(chunk)))
    return groups


GROUPS = sorted(_pattern_groups(), key=lambda t: -t[2])  # gsize desc: 3s, 2s, 1s
SLOTS = [(i, j) for (j, i0, g) in GROUPS for i in range(i0, i0 + g)]
assert len(SLOTS) == SECP and len(set(SLOTS)) == SECP

# size classes: (gsize, group_index_start, n_groups, slot_base)
SIZE_CLASSES = []
_gi = 0
_slot = 0
for _gsz in (3, 2, 1):
    _n = sum(1 for (_, _, g) in GROUPS if g == _gsz)
    if _n:
        SIZE_CLASSES.append((_gsz, _gi, _n, _slot))
        _gi += _n
        _slot += _n * _gsz
assert _slot == SECP

# tail layout: per class, groups are banded 4-per-PSUM-bank; the class's tail
# region is ceil(n/4) blocks of width gsize*OT; partition 32*band+o1t.
_tw = 0
CLASS_TAIL_BASE = []
for _gsz, _gi0, _n, _slot0 in SIZE_CLASSES:
    CLASS_TAIL_BASE.append(_tw)
    _tw += ((_n + 3) // 4) * _gsz * OT
TAILW = _tw  # tail free width per section (per partition)


def _tail_maps():
    """Static per-section tail swizzle: for (p, f) in [128, TAILW] ->
    flat element index into a section's [SECP*ROWLEN] row block, or -1."""
    fmap = np.full((128, TAILW), -1, np.int64)
    for ci, (gsz, gi0, n, slot0) in enumerate(SIZE_CLASSES):
        base = CLASS_TAIL_BASE[ci]
        for m in range(n):
            band, block = m % 4, m // 4
            for o1t in range(23):
                p = 32 * band + o1t
                for x in range(gsz):
                    slot = slot0 + m * gsz + x
                    f0 = base + block * gsz * OT + x * OT
                    fmap[p, f0 : f0 + OT] = slot * ROWLEN + (128 + o1t) * OT + np.arange(OT)
    return fmap


TAIL_FMAP = _tail_maps()
TAIL_VALID = TAIL_FMAP >= 0
TAIL_FMAP0 = np.maximum(TAIL_FMAP, 0)


def build_nc(
    nsec=NSEC,
    internal_io=False,
    null_body=False,
    loop_reps=1,
    copy_only=False,
    w_engine="vector",
    no_mm=False,
    no_add=False,
):
    """internal_io=True builds a timing variant: score buffers are Internal
    DRAM (no host transfer), with a tiny external anchor output.
    null_body=True additionally skips the whole section loop.
    loop_reps>1 wraps the body in a hardware For_i loop (timing only).
    copy_only=True strips compute: pure score DMA in/out (calibration)."""
    import concourse.bacc as bacc
    import concourse.tile as tile
    from concourse import mybir
    from contextlib import ExitStack

    f32 = mybir.dt.float32
    bf16 = mybir.dt.float16  # fp16: same PE rate as bf16, 4x less rounding

    nc = bacc.Bacc(None, target_bir_lowering=False)
    io_in = {} if internal_io else {"kind": "ExternalInput"}
    io_out = {} if internal_io else {"kind": "ExternalOutput"}
    score_main = nc.dram_tensor("score_main", [128, nsec * MAIN_W], f32, **io_in)
    score_tail = nc.dram_tensor("score_tail", [128, nsec * TAILW], f32, **io_in)
    meta = nc.dram_tensor(
        "meta", [nsec * ROWS_PER_SEC, 2 * BOX], f32, kind="ExternalInput"
    )
    iota = nc.dram_tensor("iota", [128, OT], f32, kind="ExternalInput")
    out_main = nc.dram_tensor("out_main", [128, nsec * MAIN_W], f32, **io_out)
    out_tail = nc.dram_tensor("out_tail", [128, nsec * TAILW], f32, **io_out)
    anchor = (
        nc.dram_tensor("anchor", [128, OT], f32, kind="ExternalOutput")
        if internal_io
        else None
    )

    with tile.TileContext(nc) as tc, ExitStack() as ctx:
        const_pool = ctx.enter_context(tc.tile_pool(name="const", bufs=1))
        meta_pool = ctx.enter_context(tc.tile_pool(name="meta", bufs=4))
        w_pool = ctx.enter_context(tc.tile_pool(name="w", bufs=6))
        sin_pool = ctx.enter_context(tc.tile_pool(name="sin", bufs=2))
        sout_pool = ctx.enter_context(tc.tile_pool(name="sout", bufs=2))
        tin_pool = ctx.enter_context(tc.tile_pool(name="tin", bufs=4))
        tout_pool = ctx.enter_context(tc.tile_pool(name="tout", bufs=4))
        pm_pool = ctx.enter_context(tc.tile_pool(name="pm", bufs=3, space="PSUM"))
        pt_pool = ctx.enter_context(tc.tile_pool(name="pt", bufs=2, space="PSUM"))

        iota_t = const_pool.tile([128, OT], f32)
        nc.sync.dma_start(iota_t[:], iota[:])
        zeros_t = const_pool.tile([128, 512], f32)
        nc.vector.memset(zeros_t[:], 0.0)

        meta_r = meta.rearrange("(s c r) k -> s r c k", c=2, r=128)

        if anchor is not None:
            nc.sync.dma_start(anchor[:], iota_t[:])

        import contextlib

        loop_ctx = tc.For_i(0, loop_reps, 1) if loop_reps > 1 else contextlib.nullcontext()
        with loop_ctx:
          for s in range(0 if null_body else nsec):
            if copy_only:
                for ci, (g, gi0, ngrp, slot0) in enumerate(SIZE_CLASSES):
                    cw = ngrp * g * OT
                    mbase = s * MAIN_W + slot0 * OT
                    sm = sin_pool.tile([128, cw], f32, tag=f"sin{g}")
                    nc.sync.dma_start(sm[:], score_main[:, mbase : mbase + cw])
                    nc.scalar.dma_start(out_main[:, mbase : mbase + cw], sm[:])
                    tw = ((ngrp + 3) // 4) * g * OT
                    tbase = s * TAILW + CLASS_TAIL_BASE[ci]
                    for b0 in range(0, ngrp, 4):
                        bn = min(4, ngrp - b0)
                        block = b0 // 4
                        bw = g * OT
                        hi = 32 * (bn - 1) + 23
                        fsl = slice(block * bw, (block + 1) * bw)
                        st = tin_pool.tile([128, tw], f32, tag=f"tin{g}")
                        nc.sync.dma_start(
                            st[0:hi, fsl],
                            score_tail[0:hi, tbase + block * bw :][:, 0:bw],
                        )
                        nc.scalar.dma_start(
                            out_tail[0:hi, tbase + block * bw :][:, 0:bw],
                            st[0:hi, fsl],
                        )
                continue
            mt = meta_pool.tile([128, 2, 2 * BOX], f32)
            nc.sync.dma_start(mt[:], meta_r[s])
            w_eng = getattr(nc, w_engine)
            W = []
            for c in range(2):
                w = w_pool.tile([128, BOX, OT], bf16)
                for k in range(BOX):
                    w_eng.tensor_scalar(
                        w[:, k, :],
                        iota_t[:],
                        mt[:, c, k : k + 1],
                        mt[:, c, BOX + k : BOX + k + 1],
                        mybir.AluOpType.is_equal,
                        mybir.AluOpType.mult,
                    )
                W.append(w)

            for ci, (g, gi0, ngrp, slot0) in enumerate(SIZE_CLASSES):
                cw = ngrp * g * OT  # class main width
                mbase = s * MAIN_W + slot0 * OT
                sm = sin_pool.tile([128, cw], f32, tag=f"sin{g}")
                nc.sync.dma_start(sm[:], score_main[:, mbase : mbase + cw])
                om = sout_pool.tile([128, cw], f32, tag=f"sout{g}")

                # ---- mains: clusters of <=2 groups, bank-aligned PSUM ----
                for k0 in range(0, ngrp, 2):
                    kn = min(2, ngrp - k0)
                    psm = None if no_mm else pm_pool.tile([128, kn, 512], f32, tag="pm")
                    if not no_mm:
                      for m in range(kn):
                        j, i0, _ = GROUPS[gi0 + k0 + m]
                        for c in range(2):
                            nc.tensor.matmul(
                                psm[:, m, 0 : g * OT],
                                W[c][:, j, 0:128],
                                W[c][:, i0 : i0 + g, :],
                                start=(c == 0),
                                stop=(c == 1),
                            )
                    if no_add:
                        nc.vector.tensor_copy(
                            om[:, k0 * g * OT : (k0 + kn) * g * OT],
                            sm[:, k0 * g * OT : (k0 + kn) * g * OT],
                        )
                    elif no_mm:
                        nc.vector.tensor_add(
                            om[:, k0 * g * OT : (k0 + kn) * g * OT],
                            sm[:, k0 * g * OT : (k0 + kn) * g * OT],
                            sm[:, k0 * g * OT : (k0 + kn) * g * OT],
                        )
                    else:
                     nc.vector.tensor_add(
                        om[:, k0 * g * OT : (k0 + kn) * g * OT].rearrange(
                            "p (n w) -> p n w", n=kn
                        ),
                        psm[:, :, 0 : g * OT],
                        sm[:, k0 * g * OT : (k0 + kn) * g * OT].rearrange(
                            "p (n w) -> p n w", n=kn
                        ),
                    )
                nc.scalar.dma_start(out_main[:, mbase : mbase + cw], om[:])

                # ---- tails: blocks of 4 groups banded in one PSUM bank ----
                tw = ((ngrp + 3) // 4) * g * OT  # class tail width
                tbase = s * TAILW + CLASS_TAIL_BASE[ci]
                st = tin_pool.tile([128, tw], f32, tag=f"tin{g}")
                ot = tout_pool.tile([128, tw], f32, tag=f"tout{g}")
                for b0 in range(0, ngrp, 4):
                    bn = min(4, ngrp - b0)
                    block = b0 // 4
                    bw = g * OT
                    hi = 32 * (bn - 1) + 23
                    fsl = slice(block * bw, (block + 1) * bw)
                    nc.sync.dma_start(
                        st[0:hi, fsl], score_tail[0:hi, tbase + block * bw :][:, 0:bw]
                    )
                    ptt = None if no_mm else pt_pool.tile([128, 512], f32, tag="pt")
                    if not no_mm:
                      nc.scalar.copy(ptt[0:hi, :], zeros_t[0:hi, :])
                      for m in range(bn):
                        j, i0, _ = GROUPS[gi0 + b0 + m]
                        pb = 32 * m
                        for c in range(2):
                            nc.tensor.matmul(
                                ptt[pb : pb + 23, 0 : g * OT],
                                W[c][:, j, 128:OT],
                                W[c][:, i0 : i0 + g, :],
                                start=False,
                                stop=(c == 1),
                                tile_position=(0, pb),
                                skip_group_check=True,
                            )
                    if no_add:
                        nc.vector.tensor_copy(ot[0:hi, fsl], st[0:hi, fsl])
                    elif no_mm:
                        nc.vector.tensor_add(
                            ot[0:hi, fsl], st[0:hi, fsl], st[0:hi, fsl]
                        )
                    else:
                        nc.vector.tensor_add(
                            ot[0:hi, fsl], ptt[0:hi, 0:bw], st[0:hi, fsl]
                        )
                    nc.scalar.dma_start(
                        out_tail[0:hi, tbase + block * bw :][:, 0:bw], ot[0:hi, fsl]
                    )
    return nc


# ---------------------------------------------------------------------------
# host-side routing
# ---------------------------------------------------------------------------


def _sections():
    secs = [(q, h) for q in range(NUM_QT) for h in (0, 1)]
    secs += [None] * (NCORES * NSEC - len(secs))
    return secs


def _route(obj_label, qus_type, attention, score_matrix):
    score2d = np.ascontiguousarray(score_matrix).reshape(NUM_QT * PAIR, ROWLEN)
    order = np.argsort(qus_type, kind="stable")
    counts = np.bincount(qus_type, minlength=NUM_QT)
    starts = np.concatenate([[0], np.cumsum(counts)])
    secs = _sections()

    iota_arr = np.tile(np.arange(OT, dtype=np.float32), (128, 1))
    in_maps = []
    core_rows = []  # per core: [NSEC*SECP] index into score2d or -1
    for core in range(NCORES):
        sc_rows = np.full(NSEC * SECP, -1, np.int64)
        meta = np.zeros((NSEC * ROWS_PER_SEC, 2 * BOX), np.float32)
        for sl in range(NSEC):
            sec = secs[core * NSEC + sl]
            if sec is None:
                continue
            q, h = sec
            perm = np.array([(x + 5) % 10 if h else x for x in range(BOX)])
            rows = order[starts[q] : starts[q + 1]]
            B = len(rows)
            assert B <= ROWS_PER_SEC, f"group {q} has {B} rows > {ROWS_PER_SEC}"
            meta[sl * ROWS_PER_SEC : sl * ROWS_PER_SEC + B, 0:BOX] = obj_label[rows][
                :, perm
            ].astype(np.float32)
            meta[sl * ROWS_PER_SEC : sl * ROWS_PER_SEC + B, BOX:] = attention[rows][
                :, perm
            ]
            for t, (i, j) in enumerate(SLOTS):
                I, J = perm[i], perm[j]
                p = 9 * I + (J if J < I else J - 1)
                sc_rows[sl * SECP + t] = q * PAIR + p
        full = score2d[np.maximum(sc_rows, 0)]  # [NSEC*SECP, ROWLEN]
        # main: [slot, o1<128, o2] -> [128, slot*OT]
        score_main = np.ascontiguousarray(
            full.reshape(NSEC * SECP, OT, OT)[:, :128, :]
            .transpose(1, 0, 2)
            .reshape(128, NSEC * MAIN_W)
        )
        # tail: banded swizzle per section
        fsec = full.reshape(NSEC, SECP * ROWLEN)
        score_tail = np.zeros((128, NSEC * TAILW), np.float32)
        for sl in range(NSEC):
            vals = fsec[sl][TAIL_FMAP0]
            score_tail[:, sl * TAILW : (sl + 1) * TAILW] = np.where(
                TAIL_VALID, vals, 0.0
            )
        in_maps.append(
            {
                "score_main": score_main,
                "score_tail": score_tail,
                "meta": meta,
                "iota": iota_arr,
            }
        )
        core_rows.append(sc_rows)
    return in_maps, core_rows


def _assemble(results, core_rows):
    """results: list of per-core dicts with out_main/out_tail."""
    out2d = np.empty((NUM_QT * PAIR, ROWLEN), np.float32)
    for core in range(NCORES):
        rows = core_rows[core]
        om = results[core]["out_main"]  # [128, NSEC*MAIN_W]
        ot = results[core]["out_tail"]  # [128, NSEC*TAILW]
        full = np.empty((NSEC * SECP, ROWLEN), np.float32)
        f3 = full.reshape(NSEC * SECP, OT, OT)
        f3[:, :128, :] = om.reshape(128, NSEC * SECP, OT).transpose(1, 0, 2)
        fsec = full.reshape(NSEC, SECP * ROWLEN)
        for sl in range(NSEC):
            blk = ot[:, sl * TAILW : (sl + 1) * TAILW]
            fsec[sl][TAIL_FMAP0[TAIL_VALID]] = blk[TAIL_VALID]
        mask = rows >= 0
        out2d[rows[mask]] = full[mask]
    return out2d.reshape(NUM_QT, PAIR, OT, OT)


_NC_CACHE = {}


def _get_nc(nsec):
    if nsec not in _NC_CACHE:
        nc = build_nc(nsec)
        nc.compile()
        _NC_CACHE[nsec] = nc
    return _NC_CACHE[nsec]


def kernel(obj_label, qus_type, attention, score_matrix):
    from concourse.bass_utils import run_bass_kernel_spmd

    obj_label = np.asarray(obj_label)
    qus_type = np.asarray(qus_type)
    attention = np.asarray(attention, np.float32)
    score_matrix = np.asarray(score_matrix, np.float32)

    in_maps, core_rows = _route(obj_label, qus_type, attention, score_matrix)
    nc = _get_nc(NSEC)
    res = run_bass_kernel_spmd(nc, in_maps, core_ids=list(range(NCORES)))
    return _assemble([res.results[c] for c in range(NCORES)], core_rows)
